# revision 10
# baseline (speedup 1.0000x reference)
"""Trainium2 Bass kernel for DisentangleStaticNoiseLoss (NT-Xent style loss).

Math (matches the jax reference):
    x   : [K=8192, D=128] stacked embeddings (N=8 blocks of BS=1024)
    z   : row-normalized x;  S = (z @ z.T) / 0.5
    row i (block b, sample r): positives = S[i, r + b'*BS] for b' != b,
    negatives = all j with j % BS != r.
    loss = mean over (i, pos) of [log(exp(pos) + sum_neg exp(neg)) - pos]

Sharding: 8 cores, each handles 1024 rows of S. Each core receives the FULL
x rotated so its own 1024 rows come first (host-side np.roll) -> identical
SPMD program on every core. Each core emits one fp32 partial sum; the host
adds them and divides by K*(N-1).

Engine split (the exp of K^2/8 similarities is the wall):
  The kernel stores z' = z*sqrt(2*log2e*2^23) in bf16, so the matmul PSUM
  value y = z'_i.z'_j is exactly 2^23*log2(e)*logit. Row sums of exp are
  then computed two ways, balanced across engines per supertile:
    'A' tiles: ACT exp (scale 2/A_S) with accum_out row sums.
    'D' tiles: DVE Schraudolph: int16(round((y+B32)/2^16)) IS the bf16 bit
       pattern of exp(logit); a bf16 2x-rate tensor_reduce gives row sums.
  Positives are NOT extracted from the big tiles: per m-tile one bf16 DVE
  product z'_i*z'_pair and 8 tiny ones-matmuls give the 8 positive logits
  per row directly in PSUM; one ACT exp + one DVE scale recover exp(g), g.
  rsqrt for normalization runs as DVE Newton iterations (quake seed), and
  the final log uses a calibrated bit-log on DVE, so ACT only ever needs
  the Exp table -> single ACT_TABLE_LOAD.
"""

import math
import sys

import numpy as np

if "/opt/trn_rl_repo" not in sys.path:
    sys.path.insert(0, "/opt/trn_rl_repo")

N = 8
BS = 1024
D = 128
K = N * BS          # 8192
NCORES = 8
ROWS = K // NCORES  # 1024 rows per core
MT = ROWS // 128    # 8 m-tiles of 128 rows
SUP = 2048          # PSUM supertile columns (4 banks)
NSUP = K // SUP     # 4 supertiles per m-tile

LOG2E = 1.4426950408889634
A_S = float(2**23) * LOG2E * 2.0      # psum y = A_S * (z_i . z_j); logit = 2*(z.z)
S0 = math.sqrt(A_S)                   # z' = z * S0
ACT_SCALE = 2.0 / A_S                 # exp(ACT_SCALE * y) = exp(logit)
SCHC = 404000.0                       # Schraudolph centering (calibrated)
B32 = 127.0 * float(2**23) - SCHC
INV64K = 1.0 / 65536.0
MAGICF = float(0x5F3759DF)            # quake rsqrt seed constant, as float
BLOG_B = 127.0 * float(2**23) - 53763.0  # bit-log centering (calibrated for v~8300)
BLOG_C = math.log(2.0) / float(2**23)

# per-(m,s) engine assignment for the 32 exp supertiles: 'A'=ACT exact exp,
# 'D'=DVE Schraudolph, 'P'=pool Schraudolph convert + DVE reduce.
SCHED = [["A", "D", "A", "A"] for _ in range(MT)]

_NC_CACHE = {}


def _build_nc():
    import concourse.bacc as bacc
    import concourse.bass as bass
    import concourse.tile as tile
    from concourse import mybir

    f32 = mybir.dt.float32
    bf16 = mybir.dt.bfloat16
    i16 = mybir.dt.int16
    i32 = mybir.dt.int32
    AX = mybir.AxisListType
    OP = mybir.AluOpType
    AF = mybir.ActivationFunctionType

    nc = bacc.Bacc("TRN2", target_bir_lowering=False, debug=False)
    xf = nc.declare_dram_parameter("xf", [K, D], f32, isOutput=False)
    outp = nc.declare_dram_parameter("loss_out", [1, 1], f32, isOutput=True)

    def bcast(ap, n, pos=1):
        """Insert a stride-0 dim of size n at free position `pos`."""
        dims = [list(d) for d in ap.ap]
        dims.insert(pos, [0, n])
        return bass.AP(tensor=ap.tensor, offset=ap.offset, ap=dims)

    with tile.TileContext(nc) as tc:
        with (
            tc.tile_pool(name="persist", bufs=1) as P,
            tc.tile_pool(name="work", bufs=3) as W,
            tc.tile_pool(name="dram", bufs=1, space="DRAM") as DP,
        ):
            zT = P.tile([128, K], bf16, tag="zT")        # z' transposed [D, K]
            n2all = P.tile([128, 64], f32, tag="n2all")  # row norms^2
            scall = P.tile([128, 64], f32, tag="scall")  # S0 / row norm
            fcols = P.tile([128, MT * NSUP], f32, tag="fcols")  # row-sum pieces
            Gexp = P.tile([128, 64], f32, tag="Gexp")    # exp(pos logit) (m,b)
            gl = P.tile([128, 64], f32, tag="gl")        # pos logits (m,b)
            onesb = P.tile([128, 1], bf16, tag="onesb")
            onesf = P.tile([128, 1], f32, tag="onesf")
            zdram = DP.tile([K, D], bf16, tag="zdram")

            nc.vector.memset(onesb[:], 1.0)
            nc.vector.memset(onesf[:], 1.0)

            # ---- phase A: build zT = bf16((x / ||x||).T * S0) -------------
            # Row r = g*1024 + p*8 + t lives in partition p of group-tile g
            # at index t.
            xfr = xf[:, :].rearrange("(g p t) d -> g p t d", g=8, p=128, t=8)
            zdr = zdram[:, :].rearrange("(g p t) d -> g p t d", g=8, p=128, t=8)
            xgs = []
            for g in range(8):
                # casting DMA (gpsimd SWDGE): f32 DRAM -> bf16 SBUF
                xg = W.tile([128, 8, 128], bf16, tag="xg", bufs=8)
                xgs.append(xg)
                nc.gpsimd.dma_start(out=xg[:], in_=xfr[g])
                # row norms^2: bf16 square (TensorScalarPtr 4x) + reduce
                xsqb = W.tile([128, 8, 128], bf16, tag="xsqb", bufs=2)
                nc.vector.scalar_tensor_tensor(
                    out=xsqb[:], in0=xg[:], scalar=1.0, in1=xg[:],
                    op0=OP.mult, op1=OP.mult,
                )
                nc.vector.tensor_reduce(
                    out=n2all[:, g * 8 : (g + 1) * 8], in_=xsqb[:],
                    axis=AX.X, op=OP.add,
                )
                if g in (1, 4, 7):
                    gfirst = {1: 0, 4: 2, 7: 5}[g]
                    c0, c1 = gfirst * 8, (g + 1) * 8
                    nw = c1 - c0
                    n2 = n2all[:, c0:c1]
                    # Newton rsqrt (quake seed + 1 iteration), scaled by S0
                    nf = W.tile([128, 24], f32, tag="nf", bufs=2, name="nf")[:, 0:nw]
                    nc.vector.tensor_copy(out=nf, in_=n2.bitcast(i32))
                    g0f = W.tile([128, 24], f32, tag="g0f", bufs=2, name="g0f")[:, 0:nw]
                    nc.vector.tensor_scalar(
                        out=g0f, in0=nf, scalar1=-0.5, scalar2=MAGICF,
                        op0=OP.mult, op1=OP.add,
                    )
                    g0i = W.tile([128, 24], i32, tag="g0i", bufs=2, name="g0i")[:, 0:nw]
                    nc.vector.tensor_copy(out=g0i, in_=g0f)
                    r0 = g0i.bitcast(f32)
                    tt = W.tile([128, 24], f32, tag="tt", bufs=2, name="tt")[:, 0:nw]
                    wt = W.tile([128, 24], f32, tag="wt", bufs=2, name="wt")[:, 0:nw]
                    nc.vector.tensor_tensor(out=tt, in0=n2, in1=r0, op=OP.mult)
                    nc.vector.tensor_tensor(out=tt, in0=tt, in1=r0, op=OP.mult)
                    nc.vector.tensor_scalar(
                        out=wt, in0=tt, scalar1=-0.5 * S0, scalar2=1.5 * S0,
                        op0=OP.mult, op1=OP.add,
                    )
                    nc.vector.tensor_tensor(
                        out=scall[:, c0:c1], in0=r0, in1=wt, op=OP.mult
                    )
                    for gg in range(gfirst, g + 1):
                        sc = scall[:, gg * 8 : (gg + 1) * 8]
                        scb = bcast(sc, 128, pos=2)
                        zg = W.tile([128, 8, 128], bf16, tag="zg", bufs=2)
                        nc.gpsimd.tensor_tensor(
                            out=zg[:], in0=xgs[gg][:], in1=scb, op=OP.mult
                        )
                        nc.sync.dma_start(out=zdr[gg], in_=zg[:])
                        nc.sync.dma_start_transpose(
                            out=zT[:, gg * 1024 : (gg + 1) * 1024],
                            in_=zdram[gg * 1024 : (gg + 1) * 1024, :],
                        )

            zTb = zT[:, :].rearrange("p (b r) -> p b r", b=8)

            def g_phase(PM, ms, goff):
                """Positive logits for m-tiles `ms` -> Gexp/gl[:, goff:+32]."""
                pcs = []
                for j, mm in enumerate(ms):
                    pc = W.tile([128, 8, 128], bf16, tag="pc", bufs=4)
                    pcs.append(pc)
                    # z'[d, pair_b(i)] for the 8 blocks b, i in m-tile mm
                    in0 = zTb[:, :, mm * 128 : (mm + 1) * 128]
                    in1 = bcast(zT[:, mm * 128 : (mm + 1) * 128], 8, pos=1)
                    nc.gpsimd.tensor_tensor(
                        out=pc[:], in0=in0, in1=in1, op=OP.mult
                    )
                Gp = PM.tile([128, 32], f32, tag="ps")
                for i, mm in enumerate(ms):
                    for b in range(8):
                        nc.tensor.matmul(
                            Gp[:, i * 8 + b : i * 8 + b + 1],
                            pcs[i][:, b, :],
                            onesb[:],
                            start=True,
                            stop=True,
                        )
                nc.scalar.activation(
                    out=Gexp[:, goff : goff + 32], in_=Gp[:],
                    func=AF.Exp, scale=ACT_SCALE,
                )
                nc.vector.tensor_scalar(
                    out=gl[:, goff : goff + 32], in0=Gp[:],
                    scalar1=ACT_SCALE, scalar2=None, op0=OP.mult,
                )

            # ---- phase B: S row-block supertiles, exp row sums ------------
            with tc.tile_pool(name="pmm", bufs=2, space="PSUM") as PM:
                for m in range(MT):
                    lhsT = zT[:, m * 128 : (m + 1) * 128]
                    for s in range(NSUP):
                        ps = PM.tile([128, SUP], f32, tag="ps")
                        for q in range(SUP // 512):
                            nc.tensor.matmul(
                                ps[:, q * 512 : (q + 1) * 512],
                                lhsT,
                                zT[:, s * SUP + q * 512 : s * SUP + (q + 1) * 512],
                                start=True,
                                stop=True,
                            )
                        fc = fcols[:, m * NSUP + s : m * NSUP + s + 1]
                        if SCHED[m][s] == "A":
                            dA = W.tile([128, SUP], bf16, tag="dA", bufs=2)
                            nc.scalar.activation(
                                out=dA[:], in_=ps[:], func=AF.Exp,
                                scale=ACT_SCALE, accum_out=fc,
                            )
                        else:
                            di = W.tile([128, SUP], i16, tag="di", bufs=3)
                            eng = (
                                nc.vector if SCHED[m][s] == "D" else nc.gpsimd
                            )
                            eng.tensor_scalar(
                                out=di[:], in0=ps[:], scalar1=B32,
                                scalar2=INV64K, op0=OP.add, op1=OP.mult,
                            )
                            nc.vector.tensor_reduce(
                                out=fc, in_=di[:].bitcast(bf16),
                                axis=AX.X, op=OP.add,
                            )
                    if m == 2:
                        g_phase(PM, [0, 1, 2, 3], 0)
                    if m == 5:
                        g_phase(PM, [4, 5, 6, 7], 32)

                # ---- phase C: finale ----------------------------------
                f8 = P.tile([128, MT], f32, tag="f8")
                nc.vector.tensor_reduce(
                    out=f8[:],
                    in_=fcols[:].rearrange("p (m s) -> p m s", s=NSUP),
                    axis=AX.X, op=OP.add,
                )
                p8 = P.tile([128, MT], f32, tag="p8")
                nc.vector.tensor_reduce(
                    out=p8[:],
                    in_=Gexp[:].rearrange("p (m b) -> p m b", b=8),
                    axis=AX.X, op=OP.add,
                )
                a8 = P.tile([128, MT], f32, tag="a8")
                nc.vector.tensor_sub(a8[:], f8[:], p8[:])
                tmp = P.tile([128, 7 * MT], f32, tag="tmp")
                for m in range(MT):
                    nc.vector.tensor_scalar(
                        out=tmp[:, m * 7 : (m + 1) * 7],
                        in0=Gexp[:, m * 8 + 1 : m * 8 + 8],
                        scalar1=a8[:, m : m + 1], scalar2=None, op0=OP.add,
                    )
                # bit-log: ln(v) ~ (float(bits(v)) - BLOG_B) * BLOG_C
                tif = P.tile([128, 7 * MT], f32, tag="tif")
                nc.vector.tensor_copy(out=tif[:], in_=tmp[:].bitcast(i32))
                lnt = P.tile([128, 7 * MT], f32, tag="lnt")
                nc.vector.tensor_scalar(
                    out=lnt[:], in0=tif[:], scalar1=BLOG_B, scalar2=BLOG_C,
                    op0=OP.subtract, op1=OP.mult,
                )
                waste = P.tile([128, 7 * MT], f32, tag="waste")
                rl = P.tile([128, 1], f32, tag="rl")
                nc.vector.scalar_tensor_tensor(
                    out=waste[:].rearrange("p (m b) -> p m b", b=7),
                    in0=lnt[:].rearrange("p (m b) -> p m b", b=7),
                    scalar=1.0,
                    in1=gl[:].rearrange("p (m b) -> p m b", b=8)[:, :, 1:8],
                    op0=OP.mult,
                    op1=OP.subtract,
                    accum_out=rl[:],
                )
                pf = PM.tile([1, 1], f32, tag="ps")
                nc.tensor.matmul(pf[:], rl[:], onesf[:], start=True, stop=True)
                osb = P.tile([1, 1], f32, tag="osb")
                nc.vector.tensor_copy(out=osb[:], in_=pf[:])
                nc.sync.dma_start(out=outp[:, :], in_=osb[:])

    nc.compile()
    return nc


def _get_nc():
    if "nc" not in _NC_CACHE:
        _NC_CACHE["nc"] = _build_nc()
    return _NC_CACHE["nc"]


def _make_in_maps(x):
    in_maps = []
    for c in range(NCORES):
        xc = np.ascontiguousarray(np.roll(x, -c * ROWS, axis=0))
        in_maps.append({"xf": xc})
    return in_maps


def kernel(sim: np.ndarray, _want_results: bool = False, _trace: bool = False):
    x = np.ascontiguousarray(np.asarray(sim, dtype=np.float32).reshape(K, D))
    in_maps = _make_in_maps(x)
    nc = _get_nc()
    from concourse.bass_utils import run_bass_kernel_spmd

    res = run_bass_kernel_spmd(nc, in_maps, list(range(NCORES)), trace=_trace)
    partials = np.array(
        [r["loss_out"][0, 0] for r in res.results], dtype=np.float64
    )
    loss = np.array(partials.sum() / (K * (N - 1)), dtype=np.float32)
    if _want_results:
        return loss, res
    return loss


if __name__ == "__main__":
    nc = _build_nc()
    print("build OK")


# revision 11
# speedup vs baseline: 1.0840x; 1.0840x over previous
"""Trainium2 Bass kernel for DisentangleStaticNoiseLoss (NT-Xent style loss).

Math (matches the jax reference):
    x   : [K=8192, D=128] stacked embeddings (N=8 blocks of BS=1024)
    z   : row-normalized x;  S = (z @ z.T) / 0.5
    row i (block b, sample r): positives = S[i, r + b'*BS] for b' != b,
    negatives = all j with j % BS != r.
    loss = mean over (i, pos) of [log(exp(pos) + sum_neg exp(neg)) - pos]

Sharding: 8 cores, each handles 1024 rows of S. Each core receives the FULL
x rotated so its own 1024 rows come first (host-side np.roll) -> identical
SPMD program on every core. Each core emits one fp32 partial sum; the host
adds them and divides by K*(N-1).

Engine split (the exp of K^2/8 similarities is the wall):
  The kernel stores z' = z*sqrt(2*log2e*2^23) in bf16, so the matmul PSUM
  value y = z'_i.z'_j is exactly 2^23*log2(e)*logit. Row sums of exp are
  then computed two ways, balanced across engines per supertile:
    'A' tiles: ACT exp (scale 2/A_S) with accum_out row sums.
    'D' tiles: DVE Schraudolph: int16(round((y+B32)/2^16)) IS the bf16 bit
       pattern of exp(logit); a bf16 2x-rate tensor_reduce gives row sums.
  Positives are NOT extracted from the big tiles: per m-tile one bf16 DVE
  product z'_i*z'_pair and 8 tiny ones-matmuls give the 8 positive logits
  per row directly in PSUM; one ACT exp + one DVE scale recover exp(g), g.
  rsqrt for normalization runs as DVE Newton iterations (quake seed), and
  the final log uses a calibrated bit-log on DVE, so ACT only ever needs
  the Exp table -> single ACT_TABLE_LOAD.
"""

import math
import sys

import numpy as np

if "/opt/trn_rl_repo" not in sys.path:
    sys.path.insert(0, "/opt/trn_rl_repo")

N = 8
BS = 1024
D = 128
K = N * BS          # 8192
NCORES = 8
ROWS = K // NCORES  # 1024 rows per core
MT = ROWS // 128    # 8 m-tiles of 128 rows
SUP = 2048          # PSUM supertile columns (4 banks)
NSUP = K // SUP     # 4 supertiles per m-tile

LOG2E = 1.4426950408889634
A_S = float(2**23) * LOG2E * 2.0      # psum y = A_S * (z_i . z_j); logit = 2*(z.z)
S0 = math.sqrt(A_S)                   # z' = z * S0
ACT_SCALE = 2.0 / A_S                 # exp(ACT_SCALE * y) = exp(logit)
SCHC = 404000.0                       # Schraudolph centering (calibrated)
B32 = 127.0 * float(2**23) - SCHC
INV64K = 1.0 / 65536.0
MAGICF = float(0x5F3759DF)            # quake rsqrt seed constant, as float
BLOG_B = 127.0 * float(2**23) - 53763.0  # bit-log centering (calibrated for v~8300)
BLOG_C = math.log(2.0) / float(2**23)

# per-(m,s) engine assignment for the 32 exp supertiles: 'A'=ACT exact exp,
# 'D'=DVE Schraudolph, 'P'=pool Schraudolph convert + DVE reduce.
SCHED = [
    ["A", "A", "A", "A"],
    ["A", "A", "A", "A"],
    ["A", "D", "A", "A"],
    ["A", "D", "A", "A"],
    ["A", "D", "A", "A"],
    ["A", "D", "A", "A"],
    ["A", "D", "A", "A"],
    ["A", "D", "A", "A"],
]

_NC_CACHE = {}


def _build_nc():
    import concourse.bacc as bacc
    import concourse.bass as bass
    import concourse.tile as tile
    from concourse import mybir

    f32 = mybir.dt.float32
    bf16 = mybir.dt.bfloat16
    i16 = mybir.dt.int16
    i32 = mybir.dt.int32
    AX = mybir.AxisListType
    OP = mybir.AluOpType
    AF = mybir.ActivationFunctionType

    nc = bacc.Bacc("TRN2", target_bir_lowering=False, debug=False)
    xf = nc.declare_dram_parameter("xf", [K, D], f32, isOutput=False)
    outp = nc.declare_dram_parameter("loss_out", [1, 1], f32, isOutput=True)

    def bcast(ap, n, pos=1):
        """Insert a stride-0 dim of size n at free position `pos`."""
        dims = [list(d) for d in ap.ap]
        dims.insert(pos, [0, n])
        return bass.AP(tensor=ap.tensor, offset=ap.offset, ap=dims)

    with tile.TileContext(nc) as tc:
        with (
            tc.tile_pool(name="persist", bufs=1) as P,
            tc.tile_pool(name="work", bufs=3) as W,
            tc.tile_pool(name="dram", bufs=1, space="DRAM") as DP,
        ):
            zT = P.tile([128, K], bf16, tag="zT")        # z' transposed [D, K]
            n2all = P.tile([128, 64], f32, tag="n2all")  # row norms^2
            scall = P.tile([128, 64], f32, tag="scall")  # S0 / row norm
            fcols = P.tile([128, MT * NSUP], f32, tag="fcols")  # row-sum pieces
            Gexp = P.tile([128, 64], f32, tag="Gexp")    # exp(pos logit) (m,b)
            gl = P.tile([128, 64], f32, tag="gl")        # pos logits (m,b)
            onesb = P.tile([128, 1], bf16, tag="onesb")
            onesf = P.tile([128, 1], f32, tag="onesf")
            zdram = DP.tile([K, D], bf16, tag="zdram")

            nc.vector.memset(onesb[:], 1.0)
            nc.vector.memset(onesf[:], 1.0)

            # ---- phase A: build zT = bf16((x / ||x||).T * S0) -------------
            # Row r = g*1024 + p*8 + t lives in partition p of group-tile g
            # at index t.
            xfr = xf[:, :].rearrange("(g p t) d -> g p t d", g=8, p=128, t=8)
            zdr = zdram[:, :].rearrange("(g p t) d -> g p t d", g=8, p=128, t=8)
            xgs = []
            for g in range(8):
                # casting DMA (gpsimd SWDGE): f32 DRAM -> bf16 SBUF
                xg = W.tile([128, 8, 128], bf16, tag="xg", bufs=8)
                xgs.append(xg)
                nc.gpsimd.dma_start(out=xg[:], in_=xfr[g])
                # row norms^2: bf16 square (TensorScalarPtr 4x) + reduce
                xsqb = W.tile([128, 8, 128], bf16, tag="xsqb", bufs=2)
                nc.vector.scalar_tensor_tensor(
                    out=xsqb[:], in0=xg[:], scalar=1.0, in1=xg[:],
                    op0=OP.mult, op1=OP.mult,
                )
                nc.vector.tensor_reduce(
                    out=n2all[:, g * 8 : (g + 1) * 8], in_=xsqb[:],
                    axis=AX.X, op=OP.add,
                )
                # Newton rsqrt (quake seed + 1 iteration), scaled by S0
                n2 = n2all[:, g * 8 : (g + 1) * 8]
                nf = W.tile([128, 8], f32, tag="nf", bufs=2, name="nf")
                nc.vector.tensor_copy(out=nf[:], in_=n2.bitcast(i32))
                g0f = W.tile([128, 8], f32, tag="g0f", bufs=2, name="g0f")
                nc.vector.tensor_scalar(
                    out=g0f[:], in0=nf[:], scalar1=-0.5, scalar2=MAGICF,
                    op0=OP.mult, op1=OP.add,
                )
                g0i = W.tile([128, 8], i32, tag="g0i", bufs=2, name="g0i")
                nc.vector.tensor_copy(out=g0i[:], in_=g0f[:])
                r0 = g0i.bitcast(f32)[:, :]
                tt = W.tile([128, 8], f32, tag="tt", bufs=2, name="tt")
                wt = W.tile([128, 8], f32, tag="wt", bufs=2, name="wt")
                nc.vector.tensor_tensor(out=tt[:], in0=n2, in1=r0, op=OP.mult)
                nc.vector.tensor_tensor(out=tt[:], in0=tt[:], in1=r0, op=OP.mult)
                nc.vector.tensor_scalar(
                    out=wt[:], in0=tt[:], scalar1=-0.5 * S0, scalar2=1.5 * S0,
                    op0=OP.mult, op1=OP.add,
                )
                nc.vector.tensor_tensor(
                    out=scall[:, g * 8 : (g + 1) * 8], in0=r0, in1=wt[:],
                    op=OP.mult,
                )
                sc = scall[:, g * 8 : (g + 1) * 8]
                scb = bcast(sc, 128, pos=2)
                zg = W.tile([128, 8, 128], bf16, tag="zg", bufs=2)
                nc.gpsimd.tensor_tensor(
                    out=zg[:], in0=xg[:], in1=scb, op=OP.mult
                )
                nc.sync.dma_start(out=zdr[g], in_=zg[:])
                nc.sync.dma_start_transpose(
                    out=zT[:, g * 1024 : (g + 1) * 1024],
                    in_=zdram[g * 1024 : (g + 1) * 1024, :],
                )

            zTb = zT[:, :].rearrange("p (b r) -> p b r", b=8)

            pcs = {}

            def prodcats(ms):
                for mm in ms:
                    pc = W.tile([128, 8, 128], bf16, tag="pc", bufs=8)
                    pcs[mm] = pc
                    # z'[d, pair_b(i)] for the 8 blocks b, i in m-tile mm
                    in0 = zTb[:, :, mm * 128 : (mm + 1) * 128]
                    in1 = bcast(zT[:, mm * 128 : (mm + 1) * 128], 8, pos=1)
                    nc.vector.tensor_tensor(
                        out=pc[:], in0=in0, in1=in1, op=OP.mult
                    )

            def g_phase(PM, ms, goff):
                """Positive logits for m-tiles `ms` -> Gexp/gl[:, goff:+32]."""
                Gp = PM.tile([128, 32], f32, tag="ps")
                for i, mm in enumerate(ms):
                    for b in range(8):
                        nc.tensor.matmul(
                            Gp[:, i * 8 + b : i * 8 + b + 1],
                            pcs[mm][:, b, :],
                            onesb[:],
                            start=True,
                            stop=True,
                        )
                nc.scalar.activation(
                    out=Gexp[:, goff : goff + 32], in_=Gp[:],
                    func=AF.Exp, scale=ACT_SCALE,
                )
                nc.vector.tensor_scalar(
                    out=gl[:, goff : goff + 32], in0=Gp[:],
                    scalar1=ACT_SCALE, scalar2=None, op0=OP.mult,
                )

            # ---- phase B: S row-block supertiles, exp row sums ------------
            with tc.tile_pool(name="pmm", bufs=2, space="PSUM") as PM:
                for m in range(MT):
                    lhsT = zT[:, m * 128 : (m + 1) * 128]
                    for s in range(NSUP):
                        ps = PM.tile([128, SUP], f32, tag="ps")
                        for q in range(SUP // 512):
                            nc.tensor.matmul(
                                ps[:, q * 512 : (q + 1) * 512],
                                lhsT,
                                zT[:, s * SUP + q * 512 : s * SUP + (q + 1) * 512],
                                start=True,
                                stop=True,
                            )
                        fc = fcols[:, m * NSUP + s : m * NSUP + s + 1]
                        if SCHED[m][s] == "A":
                            dA = W.tile([128, SUP], bf16, tag="dA", bufs=2)
                            nc.scalar.activation(
                                out=dA[:], in_=ps[:], func=AF.Exp,
                                scale=ACT_SCALE, accum_out=fc,
                            )
                        else:
                            di = W.tile([128, SUP], i16, tag="di", bufs=3)
                            eng = (
                                nc.vector if SCHED[m][s] == "D" else nc.gpsimd
                            )
                            eng.tensor_scalar(
                                out=di[:], in0=ps[:], scalar1=B32,
                                scalar2=INV64K, op0=OP.add, op1=OP.mult,
                            )
                            nc.vector.tensor_reduce(
                                out=fc, in_=di[:].bitcast(bf16),
                                axis=AX.X, op=OP.add,
                            )
                    if m == 2:
                        prodcats([0, 1])
                    if m == 3:
                        prodcats([2, 3])
                        g_phase(PM, [0, 1, 2, 3], 0)
                    if m == 4:
                        prodcats([4, 5])
                    if m == 5:
                        prodcats([6, 7])
                        g_phase(PM, [4, 5, 6, 7], 32)

                # ---- phase C: finale ----------------------------------
                f8 = P.tile([128, MT], f32, tag="f8")
                nc.vector.tensor_reduce(
                    out=f8[:],
                    in_=fcols[:].rearrange("p (m s) -> p m s", s=NSUP),
                    axis=AX.X, op=OP.add,
                )
                p8 = P.tile([128, MT], f32, tag="p8")
                nc.vector.tensor_reduce(
                    out=p8[:],
                    in_=Gexp[:].rearrange("p (m b) -> p m b", b=8),
                    axis=AX.X, op=OP.add,
                )
                a8 = P.tile([128, MT], f32, tag="a8")
                nc.vector.tensor_sub(a8[:], f8[:], p8[:])
                tmp = P.tile([128, 7 * MT], f32, tag="tmp")
                for m in range(MT):
                    nc.vector.tensor_scalar(
                        out=tmp[:, m * 7 : (m + 1) * 7],
                        in0=Gexp[:, m * 8 + 1 : m * 8 + 8],
                        scalar1=a8[:, m : m + 1], scalar2=None, op0=OP.add,
                    )
                # bit-log: ln(v) ~ (float(bits(v)) - BLOG_B) * BLOG_C
                tif = P.tile([128, 7 * MT], f32, tag="tif")
                nc.vector.tensor_copy(out=tif[:], in_=tmp[:].bitcast(i32))
                lnt = P.tile([128, 7 * MT], f32, tag="lnt")
                nc.vector.tensor_scalar(
                    out=lnt[:], in0=tif[:], scalar1=BLOG_B, scalar2=BLOG_C,
                    op0=OP.subtract, op1=OP.mult,
                )
                waste = P.tile([128, 7 * MT], f32, tag="waste")
                rl = P.tile([128, 1], f32, tag="rl")
                nc.vector.scalar_tensor_tensor(
                    out=waste[:].rearrange("p (m b) -> p m b", b=7),
                    in0=lnt[:].rearrange("p (m b) -> p m b", b=7),
                    scalar=1.0,
                    in1=gl[:].rearrange("p (m b) -> p m b", b=8)[:, :, 1:8],
                    op0=OP.mult,
                    op1=OP.subtract,
                    accum_out=rl[:],
                )
                pf = PM.tile([1, 1], f32, tag="ps")
                nc.tensor.matmul(pf[:], rl[:], onesf[:], start=True, stop=True)
                osb = P.tile([1, 1], f32, tag="osb")
                nc.vector.tensor_copy(out=osb[:], in_=pf[:])
                nc.sync.dma_start(out=outp[:, :], in_=osb[:])

    nc.compile()
    return nc


def _get_nc():
    if "nc" not in _NC_CACHE:
        _NC_CACHE["nc"] = _build_nc()
    return _NC_CACHE["nc"]


def _make_in_maps(x):
    in_maps = []
    for c in range(NCORES):
        xc = np.ascontiguousarray(np.roll(x, -c * ROWS, axis=0))
        in_maps.append({"xf": xc})
    return in_maps


def kernel(sim: np.ndarray, _want_results: bool = False, _trace: bool = False):
    x = np.ascontiguousarray(np.asarray(sim, dtype=np.float32).reshape(K, D))
    in_maps = _make_in_maps(x)
    nc = _get_nc()
    from concourse.bass_utils import run_bass_kernel_spmd

    res = run_bass_kernel_spmd(nc, in_maps, list(range(NCORES)), trace=_trace)
    partials = np.array(
        [r["loss_out"][0, 0] for r in res.results], dtype=np.float64
    )
    loss = np.array(partials.sum() / (K * (N - 1)), dtype=np.float32)
    if _want_results:
        return loss, res
    return loss


if __name__ == "__main__":
    nc = _build_nc()
    print("build OK")


# revision 13
# speedup vs baseline: 1.1427x; 1.0541x over previous
"""Trainium2 Bass kernel for DisentangleStaticNoiseLoss (NT-Xent style loss).

Math (matches the jax reference):
    x   : [K=8192, D=128] stacked embeddings (N=8 blocks of BS=1024)
    z   : row-normalized x;  S = (z @ z.T) / 0.5
    row i (block b, sample r): positives = S[i, r + b'*BS] for b' != b,
    negatives = all j with j % BS != r.
    loss = mean over (i, pos) of [log(exp(pos) + sum_neg exp(neg)) - pos]

Sharding: 8 cores, each handles 1024 rows of S. Each core receives the FULL
x rotated so its own 1024 rows come first (host-side np.roll) -> identical
SPMD program on every core. Each core emits one fp32 partial sum; the host
adds them and divides by K*(N-1).

Engine split (the exp of K^2/8 similarities is the wall):
  The kernel stores z' = z*sqrt(2*log2e*2^23) in bf16, so the matmul PSUM
  value y = z'_i.z'_j is exactly 2^23*log2(e)*logit. Row sums of exp are
  then computed two ways, balanced across engines per supertile:
    'A' tiles: ACT exp (scale 2/A_S) with accum_out row sums.
    'D' tiles: DVE Schraudolph: int16(round((y+B32)/2^16)) IS the bf16 bit
       pattern of exp(logit); a bf16 2x-rate tensor_reduce gives row sums.
  Positives are NOT extracted from the big tiles: per m-tile one bf16 DVE
  product z'_i*z'_pair and 8 tiny ones-matmuls give the 8 positive logits
  per row directly in PSUM; one ACT exp + one DVE scale recover exp(g), g.
  rsqrt for normalization runs as DVE Newton iterations (quake seed), and
  the final log uses a calibrated bit-log on DVE, so ACT only ever needs
  the Exp table -> single ACT_TABLE_LOAD.
"""

import math
import sys

import numpy as np

if "/opt/trn_rl_repo" not in sys.path:
    sys.path.insert(0, "/opt/trn_rl_repo")

N = 8
BS = 1024
D = 128
K = N * BS          # 8192
NCORES = 8
ROWS = K // NCORES  # 1024 rows per core
MT = ROWS // 128    # 8 m-tiles of 128 rows
SUP = 2048          # PSUM supertile columns (4 banks)
NSUP = K // SUP     # 4 supertiles per m-tile

LOG2E = 1.4426950408889634
A_S = float(2**23) * LOG2E * 2.0      # psum y = A_S * (z_i . z_j); logit = 2*(z.z)
S0 = math.sqrt(A_S)                   # z' = z * S0
ACT_SCALE = 2.0 / A_S                 # exp(ACT_SCALE * y) = exp(logit)
SCHC = 404000.0                       # Schraudolph centering (calibrated)
B32 = 127.0 * float(2**23) - SCHC
INV64K = 1.0 / 65536.0
MAGICF = float(0x5F3759DF)            # quake rsqrt seed constant, as float
BLOG_B = 127.0 * float(2**23) - 53763.0  # bit-log centering (calibrated for v~8300)
BLOG_C = math.log(2.0) / float(2**23)

# per-(m,s) engine assignment for the 32 exp supertiles: 'A'=ACT exact exp,
# 'D'=DVE Schraudolph, 'P'=pool Schraudolph convert + DVE reduce.
SCHED = [
    ["A", "A", "A", "A"],
    ["A", "D", "A", "A"],
    ["A", "D", "A", "A"],
    ["A", "D", "A", "A"],
    ["A", "D", "A", "A"],
    ["A", "D", "A", "A"],
    ["A", "D", "A", "A"],
    ["A", "D", "A", "A"],
]

_NC_CACHE = {}


def _build_nc():
    import concourse.bacc as bacc
    import concourse.bass as bass
    import concourse.tile as tile
    from concourse import mybir

    f32 = mybir.dt.float32
    bf16 = mybir.dt.bfloat16
    i16 = mybir.dt.int16
    i32 = mybir.dt.int32
    AX = mybir.AxisListType
    OP = mybir.AluOpType
    AF = mybir.ActivationFunctionType

    nc = bacc.Bacc("TRN2", target_bir_lowering=False, debug=False)
    xf = nc.declare_dram_parameter("xf", [K, D], f32, isOutput=False)
    outp = nc.declare_dram_parameter("loss_out", [1, 1], f32, isOutput=True)

    def bcast(ap, n, pos=1):
        """Insert a stride-0 dim of size n at free position `pos`."""
        dims = [list(d) for d in ap.ap]
        dims.insert(pos, [0, n])
        return bass.AP(tensor=ap.tensor, offset=ap.offset, ap=dims)

    with tile.TileContext(nc) as tc:
        with (
            tc.tile_pool(name="persist", bufs=1) as P,
            tc.tile_pool(name="work", bufs=3) as W,
            tc.tile_pool(name="dram", bufs=1, space="DRAM") as DP,
        ):
            zT = P.tile([128, K], bf16, tag="zT")        # z' transposed [D, K]
            n2all = P.tile([128, 64], f32, tag="n2all")  # row norms^2
            scall = P.tile([128, 64], f32, tag="scall")  # S0 / row norm
            fcols = P.tile([128, MT * NSUP], f32, tag="fcols")  # row-sum pieces
            Gexp = P.tile([128, 64], f32, tag="Gexp")    # exp(pos logit) (m,b)
            gl = P.tile([128, 64], f32, tag="gl")        # pos logits (m,b)
            onesb = P.tile([128, 1], bf16, tag="onesb")
            onesf = P.tile([128, 1], f32, tag="onesf")
            zdram = DP.tile([K, D], bf16, tag="zdram")

            nc.vector.memset(onesb[:], 1.0)
            nc.vector.memset(onesf[:], 1.0)

            # ---- phase A: build zT = bf16((x / ||x||).T * S0) -------------
            # Row r = g*1024 + p*8 + t lives in partition p of group-tile g
            # at index t.
            xfr = xf[:, :].rearrange("(g p t) d -> g p t d", g=8, p=128, t=8)
            zdr = zdram[:, :].rearrange("(g p t) d -> g p t d", g=8, p=128, t=8)
            xgs = []
            for g in range(8):
                xg = W.tile([128, 8, 128], f32, tag="xg", bufs=8)
                xgs.append(xg)
                nc.sync.dma_start(out=xg[:], in_=xfr[g])
                # row norms^2: square STT + reduce
                xsqb = W.tile([128, 8, 128], bf16, tag="xsqb", bufs=2)
                nc.vector.scalar_tensor_tensor(
                    out=xsqb[:], in0=xg[:], scalar=1.0, in1=xg[:],
                    op0=OP.mult, op1=OP.mult,
                )
                nc.vector.tensor_reduce(
                    out=n2all[:, g * 8 : (g + 1) * 8], in_=xsqb[:],
                    axis=AX.X, op=OP.add,
                )
                # Newton rsqrt (quake seed + 1 iteration), scaled by S0
                n2 = n2all[:, g * 8 : (g + 1) * 8]
                nf = W.tile([128, 8], f32, tag="nf", bufs=2, name="nf")
                nc.vector.tensor_copy(out=nf[:], in_=n2.bitcast(i32))
                g0f = W.tile([128, 8], f32, tag="g0f", bufs=2, name="g0f")
                nc.vector.tensor_scalar(
                    out=g0f[:], in0=nf[:], scalar1=-0.5, scalar2=MAGICF,
                    op0=OP.mult, op1=OP.add,
                )
                g0i = W.tile([128, 8], i32, tag="g0i", bufs=2, name="g0i")
                nc.vector.tensor_copy(out=g0i[:], in_=g0f[:])
                r0 = g0i.bitcast(f32)[:, :]
                tt = W.tile([128, 8], f32, tag="tt", bufs=2, name="tt")
                wt = W.tile([128, 8], f32, tag="wt", bufs=2, name="wt")
                nc.vector.tensor_tensor(out=tt[:], in0=n2, in1=r0, op=OP.mult)
                nc.vector.tensor_tensor(out=tt[:], in0=tt[:], in1=r0, op=OP.mult)
                nc.vector.tensor_scalar(
                    out=wt[:], in0=tt[:], scalar1=-0.5 * S0, scalar2=1.5 * S0,
                    op0=OP.mult, op1=OP.add,
                )
                nc.vector.tensor_tensor(
                    out=scall[:, g * 8 : (g + 1) * 8], in0=r0, in1=wt[:],
                    op=OP.mult,
                )
                sc = scall[:, g * 8 : (g + 1) * 8]
                scb = bcast(sc, 128, pos=2)
                zg = W.tile([128, 8, 128], bf16, tag="zg", bufs=2)
                nc.gpsimd.tensor_tensor(
                    out=zg[:], in0=xg[:], in1=scb, op=OP.mult
                )
                nc.sync.dma_start(out=zdr[g], in_=zg[:])
                nc.sync.dma_start_transpose(
                    out=zT[:, g * 1024 : (g + 1) * 1024],
                    in_=zdram[g * 1024 : (g + 1) * 1024, :],
                )

            zTb = zT[:, :].rearrange("p (b r) -> p b r", b=8)

            pcs = {}

            def prodcats(ms):
                for mm in ms:
                    pc = W.tile([128, 8, 128], bf16, tag="pc", bufs=8)
                    pcs[mm] = pc
                    # z'[d, pair_b(i)] for the 8 blocks b, i in m-tile mm
                    in0 = zTb[:, :, mm * 128 : (mm + 1) * 128]
                    in1 = bcast(zT[:, mm * 128 : (mm + 1) * 128], 8, pos=1)
                    nc.gpsimd.tensor_tensor(
                        out=pc[:], in0=in0, in1=in1, op=OP.mult
                    )

            def g_phase(PM, ms, goff):
                """Positive logits for m-tiles `ms` -> Gexp/gl[:, goff:+32]."""
                Gp = PM.tile([128, 32], f32, tag="ps")
                for i, mm in enumerate(ms):
                    for b in range(8):
                        nc.tensor.matmul(
                            Gp[:, i * 8 + b : i * 8 + b + 1],
                            pcs[mm][:, b, :],
                            onesb[:],
                            start=True,
                            stop=True,
                        )
                nc.scalar.activation(
                    out=Gexp[:, goff : goff + 32], in_=Gp[:],
                    func=AF.Exp, scale=ACT_SCALE,
                )
                nc.vector.tensor_scalar(
                    out=gl[:, goff : goff + 32], in0=Gp[:],
                    scalar1=ACT_SCALE, scalar2=None, op0=OP.mult,
                )

            # ---- phase B: S row-block supertiles, exp row sums ------------
            with tc.tile_pool(name="pmm", bufs=2, space="PSUM") as PM:
                for m in range(MT):
                    lhsT = zT[:, m * 128 : (m + 1) * 128]
                    for s in range(NSUP):
                        ps = PM.tile([128, SUP], f32, tag="ps")
                        for q in range(SUP // 512):
                            nc.tensor.matmul(
                                ps[:, q * 512 : (q + 1) * 512],
                                lhsT,
                                zT[:, s * SUP + q * 512 : s * SUP + (q + 1) * 512],
                                start=True,
                                stop=True,
                            )
                        fc = fcols[:, m * NSUP + s : m * NSUP + s + 1]
                        if SCHED[m][s] == "A":
                            dA = W.tile([128, SUP], bf16, tag="dA", bufs=2)
                            nc.scalar.activation(
                                out=dA[:], in_=ps[:], func=AF.Exp,
                                scale=ACT_SCALE, accum_out=fc,
                            )
                        else:
                            di = W.tile([128, SUP], i16, tag="di", bufs=3)
                            eng = (
                                nc.vector if SCHED[m][s] == "D" else nc.gpsimd
                            )
                            eng.tensor_scalar(
                                out=di[:], in0=ps[:], scalar1=B32,
                                scalar2=INV64K, op0=OP.add, op1=OP.mult,
                            )
                            nc.vector.tensor_reduce(
                                out=fc, in_=di[:].bitcast(bf16),
                                axis=AX.X, op=OP.add,
                            )
                    if m == 2:
                        prodcats([0, 1])
                    if m == 3:
                        prodcats([2, 3])
                    if m == 4:
                        prodcats([4, 5])
                    if m == 5:
                        prodcats([6, 7])
                        g_phase(PM, [0, 1, 2, 3], 0)
                    if m == 6:
                        g_phase(PM, [4, 5, 6, 7], 32)

                # ---- phase C: finale ----------------------------------
                f8 = P.tile([128, MT], f32, tag="f8")
                nc.vector.tensor_reduce(
                    out=f8[:],
                    in_=fcols[:].rearrange("p (m s) -> p m s", s=NSUP),
                    axis=AX.X, op=OP.add,
                )
                p8 = P.tile([128, MT], f32, tag="p8")
                nc.vector.tensor_reduce(
                    out=p8[:],
                    in_=Gexp[:].rearrange("p (m b) -> p m b", b=8),
                    axis=AX.X, op=OP.add,
                )
                a8 = P.tile([128, MT], f32, tag="a8")
                nc.vector.tensor_sub(a8[:], f8[:], p8[:])
                tmp = P.tile([128, 7 * MT], f32, tag="tmp")
                for m in range(MT):
                    nc.vector.tensor_scalar(
                        out=tmp[:, m * 7 : (m + 1) * 7],
                        in0=Gexp[:, m * 8 + 1 : m * 8 + 8],
                        scalar1=a8[:, m : m + 1], scalar2=None, op0=OP.add,
                    )
                # bit-log: ln(v) ~ (float(bits(v)) - BLOG_B) * BLOG_C
                tif = P.tile([128, 7 * MT], f32, tag="tif")
                nc.vector.tensor_copy(out=tif[:], in_=tmp[:].bitcast(i32))
                lnt = P.tile([128, 7 * MT], f32, tag="lnt")
                nc.vector.tensor_scalar(
                    out=lnt[:], in0=tif[:], scalar1=BLOG_B, scalar2=BLOG_C,
                    op0=OP.subtract, op1=OP.mult,
                )
                waste = P.tile([128, 7 * MT], f32, tag="waste")
                rl = P.tile([128, 1], f32, tag="rl")
                nc.vector.scalar_tensor_tensor(
                    out=waste[:].rearrange("p (m b) -> p m b", b=7),
                    in0=lnt[:].rearrange("p (m b) -> p m b", b=7),
                    scalar=1.0,
                    in1=gl[:].rearrange("p (m b) -> p m b", b=8)[:, :, 1:8],
                    op0=OP.mult,
                    op1=OP.subtract,
                    accum_out=rl[:],
                )
                pf = PM.tile([1, 1], f32, tag="ps")
                nc.tensor.matmul(pf[:], rl[:], onesf[:], start=True, stop=True)
                osb = P.tile([1, 1], f32, tag="osb")
                nc.vector.tensor_copy(out=osb[:], in_=pf[:])
                nc.sync.dma_start(out=outp[:, :], in_=osb[:])

    nc.compile()
    return nc


def _get_nc():
    if "nc" not in _NC_CACHE:
        _NC_CACHE["nc"] = _build_nc()
    return _NC_CACHE["nc"]


def _make_in_maps(x):
    in_maps = []
    for c in range(NCORES):
        xc = np.ascontiguousarray(np.roll(x, -c * ROWS, axis=0))
        in_maps.append({"xf": xc})
    return in_maps


def kernel(sim: np.ndarray, _want_results: bool = False, _trace: bool = False):
    x = np.ascontiguousarray(np.asarray(sim, dtype=np.float32).reshape(K, D))
    in_maps = _make_in_maps(x)
    nc = _get_nc()
    from concourse.bass_utils import run_bass_kernel_spmd

    res = run_bass_kernel_spmd(nc, in_maps, list(range(NCORES)), trace=_trace)
    partials = np.array(
        [r["loss_out"][0, 0] for r in res.results], dtype=np.float64
    )
    loss = np.array(partials.sum() / (K * (N - 1)), dtype=np.float32)
    if _want_results:
        return loss, res
    return loss


if __name__ == "__main__":
    nc = _build_nc()
    print("build OK")


# revision 14
# speedup vs baseline: 1.1693x; 1.0232x over previous
"""Trainium2 Bass kernel for DisentangleStaticNoiseLoss (NT-Xent style loss).

Math (matches the jax reference):
    x   : [K=8192, D=128] stacked embeddings (N=8 blocks of BS=1024)
    z   : row-normalized x;  S = (z @ z.T) / 0.5
    row i (block b, sample r): positives = S[i, r + b'*BS] for b' != b,
    negatives = all j with j % BS != r.
    loss = mean over (i, pos) of [log(exp(pos) + sum_neg exp(neg)) - pos]

Sharding: 8 cores, each handles 1024 rows of S. Each core receives the FULL
x rotated so its own 1024 rows come first (host-side np.roll) -> identical
SPMD program on every core. Each core emits one fp32 partial sum; the host
adds them and divides by K*(N-1).

Engine split (the exp of K^2/8 similarities is the wall):
  The kernel stores z' = z*sqrt(2*log2e*2^23) in bf16, so the matmul PSUM
  value y = z'_i.z'_j is exactly 2^23*log2(e)*logit. Row sums of exp are
  then computed two ways, balanced across engines per supertile:
    'A' tiles: ACT exp (scale 2/A_S) with accum_out row sums.
    'D' tiles: DVE Schraudolph: int16(round((y+B32)/2^16)) IS the bf16 bit
       pattern of exp(logit); a bf16 2x-rate tensor_reduce gives row sums.
  Positives are NOT extracted from the big tiles: per m-tile one bf16 DVE
  product z'_i*z'_pair and 8 tiny ones-matmuls give the 8 positive logits
  per row directly in PSUM; one ACT exp + one DVE scale recover exp(g), g.
  rsqrt for normalization runs as DVE Newton iterations (quake seed), and
  the final log uses a calibrated bit-log on DVE, so ACT only ever needs
  the Exp table -> single ACT_TABLE_LOAD.
"""

import math
import sys

import numpy as np

if "/opt/trn_rl_repo" not in sys.path:
    sys.path.insert(0, "/opt/trn_rl_repo")

N = 8
BS = 1024
D = 128
K = N * BS          # 8192
NCORES = 8
ROWS = K // NCORES  # 1024 rows per core
MT = ROWS // 128    # 8 m-tiles of 128 rows
SUP = 2048          # PSUM supertile columns (4 banks)
NSUP = K // SUP     # 4 supertiles per m-tile

LOG2E = 1.4426950408889634
A_S = float(2**23) * LOG2E * 2.0      # psum y = A_S * (z_i . z_j); logit = 2*(z.z)
S0 = math.sqrt(A_S)                   # z' = z * S0
ACT_SCALE = 2.0 / A_S                 # exp(ACT_SCALE * y) = exp(logit)
SCHC = 404000.0                       # Schraudolph centering (calibrated)
B32 = 127.0 * float(2**23) - SCHC
INV64K = 1.0 / 65536.0
MAGICF = float(0x5F3759DF)            # quake rsqrt seed constant, as float
BLOG_B = 127.0 * float(2**23) - 53763.0  # bit-log centering (calibrated for v~8300)
BLOG_C = math.log(2.0) / float(2**23)

# per-(m,s) engine assignment for the 32 exp supertiles: 'A'=ACT exact exp,
# 'D'=DVE Schraudolph, 'P'=pool Schraudolph convert + DVE reduce.
SCHED = [
    ["A", "A", "A", "A"],
    ["A", "A", "A", "A"],
    ["A", "A", "A", "A"],
    ["A", "A", "D", "A"],
    ["A", "A", "D", "A"],
    ["A", "A", "D", "A"],
    ["A", "A", "D", "A"],
    ["A", "A", "D", "A"],
]

_NC_CACHE = {}


def _build_nc():
    import concourse.bacc as bacc
    import concourse.bass as bass
    import concourse.tile as tile
    from concourse import mybir

    f32 = mybir.dt.float32
    bf16 = mybir.dt.bfloat16
    i16 = mybir.dt.int16
    i32 = mybir.dt.int32
    AX = mybir.AxisListType
    OP = mybir.AluOpType
    AF = mybir.ActivationFunctionType

    nc = bacc.Bacc("TRN2", target_bir_lowering=False, debug=False)
    xf = nc.declare_dram_parameter("xf", [K, D], f32, isOutput=False)
    outp = nc.declare_dram_parameter("loss_out", [1, 1], f32, isOutput=True)

    def bcast(ap, n, pos=1):
        """Insert a stride-0 dim of size n at free position `pos`."""
        dims = [list(d) for d in ap.ap]
        dims.insert(pos, [0, n])
        return bass.AP(tensor=ap.tensor, offset=ap.offset, ap=dims)

    with tile.TileContext(nc) as tc:
        with (
            tc.tile_pool(name="persist", bufs=1) as P,
            tc.tile_pool(name="work", bufs=3) as W,
            tc.tile_pool(name="dram", bufs=1, space="DRAM") as DP,
        ):
            zT = P.tile([128, K], bf16, tag="zT")        # z' transposed [D, K]
            n2all = P.tile([128, 64], f32, tag="n2all")  # row norms^2
            scall = P.tile([128, 64], f32, tag="scall")  # S0 / row norm
            fcols = P.tile([128, MT * NSUP], f32, tag="fcols")  # row-sum pieces
            Gexp = P.tile([128, 64], f32, tag="Gexp")    # exp(pos logit) (m,b)
            gl = P.tile([128, 64], f32, tag="gl")        # pos logits (m,b)
            onesb = P.tile([128, 1], bf16, tag="onesb")
            onesf = P.tile([128, 1], f32, tag="onesf")
            zdram = DP.tile([K, D], bf16, tag="zdram")

            nc.vector.memset(onesb[:], 1.0)
            nc.vector.memset(onesf[:], 1.0)

            # ---- phase A: build zT = bf16((x / ||x||).T * S0) -------------
            # Row r = g*1024 + p*8 + t lives in partition p of group-tile g
            # at index t.
            xfr = xf[:, :].rearrange("(g p t) d -> g p t d", g=8, p=128, t=8)
            zdr = zdram[:, :].rearrange("(g p t) d -> g p t d", g=8, p=128, t=8)
            xgs = []
            for g in range(8):
                xg = W.tile([128, 8, 128], f32, tag="xg", bufs=8)
                xgs.append(xg)
                nc.sync.dma_start(out=xg[:], in_=xfr[g])
                # row norms^2: square STT + reduce
                xsqb = W.tile([128, 8, 128], bf16, tag="xsqb", bufs=2)
                nc.vector.scalar_tensor_tensor(
                    out=xsqb[:], in0=xg[:], scalar=1.0, in1=xg[:],
                    op0=OP.mult, op1=OP.mult,
                )
                nc.vector.tensor_reduce(
                    out=n2all[:, g * 8 : (g + 1) * 8], in_=xsqb[:],
                    axis=AX.X, op=OP.add,
                )
                # Newton rsqrt (quake seed + 1 iteration), scaled by S0
                n2 = n2all[:, g * 8 : (g + 1) * 8]
                nf = W.tile([128, 8], f32, tag="nf", bufs=2, name="nf")
                nc.vector.tensor_copy(out=nf[:], in_=n2.bitcast(i32))
                g0f = W.tile([128, 8], f32, tag="g0f", bufs=2, name="g0f")
                nc.vector.tensor_scalar(
                    out=g0f[:], in0=nf[:], scalar1=-0.5, scalar2=MAGICF,
                    op0=OP.mult, op1=OP.add,
                )
                g0i = W.tile([128, 8], i32, tag="g0i", bufs=2, name="g0i")
                nc.vector.tensor_copy(out=g0i[:], in_=g0f[:])
                r0 = g0i.bitcast(f32)[:, :]
                tt = W.tile([128, 8], f32, tag="tt", bufs=2, name="tt")
                wt = W.tile([128, 8], f32, tag="wt", bufs=2, name="wt")
                nc.vector.tensor_tensor(out=tt[:], in0=n2, in1=r0, op=OP.mult)
                nc.vector.tensor_tensor(out=tt[:], in0=tt[:], in1=r0, op=OP.mult)
                nc.vector.tensor_scalar(
                    out=wt[:], in0=tt[:], scalar1=-0.5 * S0, scalar2=1.5 * S0,
                    op0=OP.mult, op1=OP.add,
                )
                nc.vector.tensor_tensor(
                    out=scall[:, g * 8 : (g + 1) * 8], in0=r0, in1=wt[:],
                    op=OP.mult,
                )
                sc = scall[:, g * 8 : (g + 1) * 8]
                scb = bcast(sc, 128, pos=2)
                zg = W.tile([128, 8, 128], bf16, tag="zg", bufs=2)
                nc.gpsimd.tensor_tensor(
                    out=zg[:], in0=xg[:], in1=scb, op=OP.mult
                )
                nc.sync.dma_start(out=zdr[g], in_=zg[:])
                nc.sync.dma_start_transpose(
                    out=zT[:, g * 1024 : (g + 1) * 1024],
                    in_=zdram[g * 1024 : (g + 1) * 1024, :],
                )

            zTb = zT[:, :].rearrange("p (b r) -> p b r", b=8)

            pcs = {}

            def prodcats(ms):
                for mm in ms:
                    pc = W.tile([128, 8, 128], bf16, tag="pc", bufs=8)
                    pcs[mm] = pc
                    # z'[d, pair_b(i)] for the 8 blocks b, i in m-tile mm
                    in0 = zTb[:, :, mm * 128 : (mm + 1) * 128]
                    in1 = bcast(zT[:, mm * 128 : (mm + 1) * 128], 8, pos=1)
                    nc.gpsimd.tensor_tensor(
                        out=pc[:], in0=in0, in1=in1, op=OP.mult
                    )

            def g_phase(PM, ms, goff):
                """Positive logits for m-tiles `ms` -> Gexp/gl[:, goff:+32]."""
                Gp = PM.tile([128, 32], f32, tag="ps")
                for i, mm in enumerate(ms):
                    for b in range(8):
                        nc.tensor.matmul(
                            Gp[:, i * 8 + b : i * 8 + b + 1],
                            pcs[mm][:, b, :],
                            onesb[:],
                            start=True,
                            stop=True,
                        )
                nc.scalar.activation(
                    out=Gexp[:, goff : goff + 32], in_=Gp[:],
                    func=AF.Exp, scale=ACT_SCALE,
                )
                nc.vector.tensor_scalar(
                    out=gl[:, goff : goff + 32], in0=Gp[:],
                    scalar1=ACT_SCALE, scalar2=None, op0=OP.mult,
                )

            # ---- phase B: S row-block supertiles, exp row sums ------------
            with tc.tile_pool(name="pmm", bufs=2, space="PSUM") as PM:
                for m in range(MT):
                    lhsT = zT[:, m * 128 : (m + 1) * 128]
                    for s in range(NSUP):
                        ps = PM.tile([128, SUP], f32, tag="ps")
                        for q in range(SUP // 512):
                            nc.tensor.matmul(
                                ps[:, q * 512 : (q + 1) * 512],
                                lhsT,
                                zT[:, s * SUP + q * 512 : s * SUP + (q + 1) * 512],
                                start=True,
                                stop=True,
                            )
                        fc = fcols[:, m * NSUP + s : m * NSUP + s + 1]
                        if SCHED[m][s] == "A":
                            dA = W.tile([128, SUP], bf16, tag="dA", bufs=2)
                            nc.scalar.activation(
                                out=dA[:], in_=ps[:], func=AF.Exp,
                                scale=ACT_SCALE, accum_out=fc,
                            )
                        else:
                            di = W.tile([128, SUP], i16, tag="di", bufs=3)
                            eng = (
                                nc.vector if SCHED[m][s] == "D" else nc.gpsimd
                            )
                            eng.tensor_scalar(
                                out=di[:], in0=ps[:], scalar1=B32,
                                scalar2=INV64K, op0=OP.add, op1=OP.mult,
                            )
                            nc.vector.tensor_reduce(
                                out=fc, in_=di[:].bitcast(bf16),
                                axis=AX.X, op=OP.add,
                            )
                    if m == 2:
                        prodcats([0, 1])
                    if m == 3:
                        prodcats([2, 3])
                    if m == 4:
                        prodcats([4, 5])
                    if m == 5:
                        prodcats([6, 7])
                        g_phase(PM, [0, 1, 2, 3], 0)
                    if m == 6:
                        g_phase(PM, [4, 5, 6, 7], 32)

                # ---- phase C: finale ----------------------------------
                f8 = P.tile([128, MT], f32, tag="f8")
                nc.vector.tensor_reduce(
                    out=f8[:],
                    in_=fcols[:].rearrange("p (m s) -> p m s", s=NSUP),
                    axis=AX.X, op=OP.add,
                )
                p8 = P.tile([128, MT], f32, tag="p8")
                nc.vector.tensor_reduce(
                    out=p8[:],
                    in_=Gexp[:].rearrange("p (m b) -> p m b", b=8),
                    axis=AX.X, op=OP.add,
                )
                a8 = P.tile([128, MT], f32, tag="a8")
                nc.vector.tensor_sub(a8[:], f8[:], p8[:])
                tmp = P.tile([128, 7 * MT], f32, tag="tmp")
                for m in range(MT):
                    nc.vector.tensor_scalar(
                        out=tmp[:, m * 7 : (m + 1) * 7],
                        in0=Gexp[:, m * 8 + 1 : m * 8 + 8],
                        scalar1=a8[:, m : m + 1], scalar2=None, op0=OP.add,
                    )
                # bit-log: ln(v) ~ (float(bits(v)) - BLOG_B) * BLOG_C
                tif = P.tile([128, 7 * MT], f32, tag="tif")
                nc.vector.tensor_copy(out=tif[:], in_=tmp[:].bitcast(i32))
                lnt = P.tile([128, 7 * MT], f32, tag="lnt")
                nc.vector.tensor_scalar(
                    out=lnt[:], in0=tif[:], scalar1=BLOG_B, scalar2=BLOG_C,
                    op0=OP.subtract, op1=OP.mult,
                )
                waste = P.tile([128, 7 * MT], f32, tag="waste")
                rl = P.tile([128, 1], f32, tag="rl")
                nc.vector.scalar_tensor_tensor(
                    out=waste[:].rearrange("p (m b) -> p m b", b=7),
                    in0=lnt[:].rearrange("p (m b) -> p m b", b=7),
                    scalar=1.0,
                    in1=gl[:].rearrange("p (m b) -> p m b", b=8)[:, :, 1:8],
                    op0=OP.mult,
                    op1=OP.subtract,
                    accum_out=rl[:],
                )
                pf = PM.tile([1, 1], f32, tag="ps")
                nc.tensor.matmul(pf[:], rl[:], onesf[:], start=True, stop=True)
                osb = P.tile([1, 1], f32, tag="osb")
                nc.vector.tensor_copy(out=osb[:], in_=pf[:])
                nc.sync.dma_start(out=outp[:, :], in_=osb[:])

    nc.compile()
    return nc


def _get_nc():
    if "nc" not in _NC_CACHE:
        _NC_CACHE["nc"] = _build_nc()
    return _NC_CACHE["nc"]


def _make_in_maps(x):
    in_maps = []
    for c in range(NCORES):
        xc = np.ascontiguousarray(np.roll(x, -c * ROWS, axis=0))
        in_maps.append({"xf": xc})
    return in_maps


def kernel(sim: np.ndarray, _want_results: bool = False, _trace: bool = False):
    x = np.ascontiguousarray(np.asarray(sim, dtype=np.float32).reshape(K, D))
    in_maps = _make_in_maps(x)
    nc = _get_nc()
    from concourse.bass_utils import run_bass_kernel_spmd

    res = run_bass_kernel_spmd(nc, in_maps, list(range(NCORES)), trace=_trace)
    partials = np.array(
        [r["loss_out"][0, 0] for r in res.results], dtype=np.float64
    )
    loss = np.array(partials.sum() / (K * (N - 1)), dtype=np.float32)
    if _want_results:
        return loss, res
    return loss


if __name__ == "__main__":
    nc = _build_nc()
    print("build OK")
